# revision 1
# baseline (speedup 1.0000x reference)
"""Causal self-attention on 8 trn2 NeuronCores.

Sharding: core c -> (batch b = c//2, head-group g = c%2 of 8 heads).
Each core computes qkv for its (batch, head-group), causal attention for
its 8 heads, and the row-slice of the output projection for its 512
channels. Host sums the two per-batch partial projections.

Kernel design:
- x is passed transposed per batch (xT [1024, 2048]) so the contraction
  dim (model channels) lands on SBUF partitions for all qkv matmuls.
- Scores are computed transposed (S^T [keys, queries]): softmax
  denominator comes from a ones-column appended to V inside the PV
  matmul; normalization is applied to the unnormalized head outputs
  (fast reciprocal + gpsimd partition-broadcast + one multiply).
- Head-PAIR packing: the two heads of a feature tile occupy PE row
  groups 0-63 / 64-127; their K=64 score matmuls are emitted adjacently
  so the PE runs them concurrently in different row groups (~2x).
  One wide exp [128, 1024] covers both heads' score blocks.
- Causal: key-tile blocks below the diagonal run unmasked, blocks above
  are skipped, the 4 diagonal blocks per query macro get a 0/1
  multiplicative mask post-exp.
- All matmuls in float32r (single-pass reduced fp32, ~250ns/128x128x512).
"""

import sys

sys.path.insert(0, "/opt/trn_rl_repo")

import numpy as np
import ml_dtypes

import concourse.bass as bass
import concourse.mybir as mybir
import concourse.tile as tile
from concourse import bacc
from concourse.bass_utils import run_bass_kernel_spmd

# Problem shape (hardcoded per the contract).
B = 4
T = 2048
C = 1024
N_HEADS = 16
HD = 64
N_CORES = 8

# Per-core sharding.
H_PER_CORE = 8          # heads per core
CH = H_PER_CORE * HD    # 512 channels per core
KC = C // 128           # 8 contraction tiles over model dim
FT = CH * 2 // 128      # 8 feature tiles for q+k ([q0..q3, k0..k3])
TT = T // 128           # 16 token tiles
QM = T // 512           # 4 query macros
NQ = 4                  # token quarters in phase Q
SCALE = HD ** -0.5

F32 = mybir.dt.float32
F32R = mybir.dt.float32r
BF16 = mybir.dt.bfloat16

_CACHE = {}


def build_kernel(debug=False):
    nc = bacc.Bacc(target_bir_lowering=False)

    xT = nc.dram_tensor("xT", [C, T], F32R, kind="ExternalInput")
    w_qk = nc.dram_tensor("w_qk", [FT, 128, KC, 128], F32R, kind="ExternalInput")
    w_v = nc.dram_tensor("w_v", [KC, 128, CH], F32R, kind="ExternalInput")
    w_pj = nc.dram_tensor("w_pj", [CH // 128, 128, C], F32R, kind="ExternalInput")
    masks = nc.dram_tensor("masks", [128, 4, 1024], F32R, kind="ExternalInput")
    ones_d = nc.dram_tensor("ones_d", [128, 64], F32R, kind="ExternalInput")
    y = nc.dram_tensor("y", [T, C], F32, kind="ExternalOutput")
    if debug:
        dbg_qk = nc.dram_tensor("dbg_qk", [FT, 128, T], F32R, kind="ExternalOutput")
        dbg_v = nc.dram_tensor("dbg_v", [TT, 128, H_PER_CORE, HD + 1], F32R,
                               kind="ExternalOutput")
        dbg_o = nc.dram_tensor("dbg_o", [CH // 128, 128, T], F32R,
                               kind="ExternalOutput")

    with tile.TileContext(nc) as tc:
        with tc.tile_pool(name="big", bufs=1) as big:
            # ---- resident SBUF tensors ----
            qkT = [big.tile([128, T], F32R, tag=f"qkT{ft}", name=f"qkT{ft}")
                   for ft in range(FT)]
            vt = [big.tile([128, H_PER_CORE, HD + 1], F32R, tag=f"v{tt}",
                           name=f"v{tt}") for tt in range(TT)]
            outT = [big.tile([128, T], F32R, tag=f"outT{ct}", name=f"outT{ct}")
                    for ct in range(CH // 128)]
            ones_sb = big.tile([1, HD], F32R, tag="ones")

            nc.sync.dma_start(out=ones_sb, in_=ones_d[0:1, 0:HD])
            ones_col = ones_d[:, 0:H_PER_CORE].rearrange("p (a b) -> p a b", b=1)
            for tt in range(TT):
                nc.sync.dma_start(out=vt[tt][:, :, HD:HD + 1], in_=ones_col)

            # ---- phase Q: qkv projections, one token quarter at a time ----
            with (
                tc.tile_pool(name="xtp", bufs=2) as xtp,
                tc.tile_pool(name="wqs", bufs=2) as wqs,
                tc.tile_pool(name="wvs", bufs=3) as wvs,
                tc.tile_pool(name="psq", bufs=3, space="PSUM") as psq,
                tc.tile_pool(name="psv", bufs=1, space="PSUM") as psv,
            ):
                for tq in range(NQ):
                    xts = []
                    for kc in range(KC):
                        xt = xtp.tile([128, 512], F32R, tag=f"xt{kc}",
                                      name=f"xt{kc}", bufs=2)
                        nc.sync.dma_start(
                            out=xt, in_=xT[kc * 128:(kc + 1) * 128,
                                           tq * 512:(tq + 1) * 512])
                        xts.append(xt)

                    # q^T / k^T: [feat, tok] tiles
                    for ft in range(FT):
                        wq8 = wqs.tile([128, KC, 128], F32R, tag="wq")
                        nc.sync.dma_start(out=wq8, in_=w_qk[ft])
                        acc = psq.tile([128, 512], F32, tag="qk")
                        for kc in range(KC):
                            nc.tensor.matmul(
                                acc[:], wq8[:, kc, :], xts[kc][:],
                                start=(kc == 0), stop=(kc == KC - 1))
                        nc.scalar.copy(
                            qkT[ft][:, tq * 512:(tq + 1) * 512], acc[:])

                    # v: [tok, feat] tiles + ones col; kc-outer so w_v
                    # streams once per quarter, 4 token-tile psums live
                    vaccs = [psv.tile([128, CH], F32, tag=f"v{i}", name=f"vac{i}")
                             for i in range(4)]
                    for kc in range(KC):
                        wv = wvs.tile([128, CH], F32R, tag="wv")
                        nc.sync.dma_start(out=wv, in_=w_v[kc])
                        for i in range(4):
                            tt = tq * 4 + i
                            nc.tensor.matmul(
                                vaccs[i][:],
                                xts[kc][:, i * 128:(i + 1) * 128],
                                wv[:],
                                start=(kc == 0), stop=(kc == KC - 1))
                    for i in range(4):
                        tt = tq * 4 + i
                        nc.scalar.copy(
                            vt[tt][:, :, 0:HD],
                            vaccs[i][:].rearrange("p (h d) -> p h d",
                                                  h=H_PER_CORE))

            # ---- phase A: causal attention, head pairs packed ----
            with (
                tc.tile_pool(name="pts", bufs=4) as pts,
                tc.tile_pool(name="sml", bufs=4) as sml,
                tc.tile_pool(name="msk", bufs=1) as mskp,
                tc.tile_pool(name="pssw", bufs=2, space="PSUM") as pssw,
                tc.tile_pool(name="pso", bufs=2, space="PSUM") as pso,
            ):
                mask_sb = mskp.tile([128, 4, 1024], F32R, tag="masks")
                nc.sync.dma_start(out=mask_sb, in_=masks[:])
                for p in range(4):          # head pair = heads 2p, 2p+1
                    qTh = qkT[p]
                    kTh = qkT[4 + p]
                    for qm in range(QM):
                        nkt = 4 * qm + 4
                        oacc = [pso.tile([HD + 1, 512], F32, tag=f"o{hh}",
                                         name=f"o{hh}") for hh in range(2)]
                        for kt in range(nkt):
                            j = kt - 4 * qm     # >=0 on diagonal blocks
                            o0 = max(j, 0) * 128   # first valid query col
                            sw = pssw.tile([128, 1024], F32, tag="sw")
                            for hh in range(2):
                                nc.tensor.matmul(
                                    sw[:, hh * 512 + o0:(hh + 1) * 512],
                                    kTh[hh * 64:(hh + 1) * 64,
                                        kt * 128:(kt + 1) * 128],
                                    qTh[hh * 64:(hh + 1) * 64,
                                        qm * 512 + o0:(qm + 1) * 512],
                                    start=True, stop=True)
                            pt = pts.tile([128, 1024], F32R, tag="pT")
                            swv = sw[:].rearrange("p (a q) -> p a q", a=2)
                            ptv = pt[:].rearrange("p (a q) -> p a q", a=2)
                            nc.scalar.activation(
                                ptv[:, :, o0:512], swv[:, :, o0:512],
                                mybir.ActivationFunctionType.Exp, scale=SCALE)
                            if j >= 0:      # diagonal block: 0/1 mask both heads
                                mv = mask_sb[:, j, :].rearrange(
                                    "p (a q) -> p a q", a=2)
                                nc.vector.tensor_mul(
                                    ptv[:, :, o0:512], ptv[:, :, o0:512],
                                    mv[:, :, o0:512])
                            for hh in range(2):
                                h = 2 * p + hh
                                nc.tensor.matmul(
                                    oacc[hh][:, o0:512],
                                    vt[kt][:, h, :],
                                    pt[:, hh * 512 + o0:(hh + 1) * 512],
                                    start=(kt == 0), stop=(kt == nkt - 1),
                                    skip_group_check=True)
                        for hh in range(2):
                            den = sml.tile([1, 512], F32, tag="den")
                            nc.vector.tensor_copy(den[:], oacc[hh][HD:HD + 1, :])
                            rd = sml.tile([1, 512], F32, tag="rd")
                            nc.vector.reciprocal_approx_fast(rd[:], den[:])
                            bcs = sml.tile([HD, 512], F32, tag="bcs")
                            nc.gpsimd.partition_broadcast(bcs[:], rd[:])
                            nc.vector.tensor_mul(
                                outT[p][hh * 64:(hh + 1) * 64,
                                        qm * 512:(qm + 1) * 512],
                                oacc[hh][0:HD, :], bcs[:])

            if debug:
                for ft in range(FT):
                    nc.sync.dma_start(out=dbg_qk[ft], in_=qkT[ft][:])
                for tt in range(TT):
                    nc.sync.dma_start(out=dbg_v[tt], in_=vt[tt][:])
                for ct in range(CH // 128):
                    nc.sync.dma_start(out=dbg_o[ct], in_=outT[ct][:])

            # ---- phase P: output projection (row-parallel slice) ----
            with (
                tc.tile_pool(name="wps", bufs=2) as wps,
                tc.tile_pool(name="ysb", bufs=3) as ysbp,
                tc.tile_pool(name="psp", bufs=3, space="PSUM") as psp,
            ):
                for nf in range(2):
                    wpj = []
                    for ct in range(CH // 128):
                        w = wps.tile([128, 512], F32R, tag=f"wpj{ct}",
                                     name=f"wpj{ct}")
                        nc.sync.dma_start(
                            out=w, in_=w_pj[ct][:, nf * 512:(nf + 1) * 512])
                        wpj.append(w)
                    for tt in range(TT):
                        accp = psp.tile([128, 512], F32, tag="pp")
                        for ct in range(CH // 128):
                            nc.tensor.matmul(
                                accp[:],
                                outT[ct][:, tt * 128:(tt + 1) * 128],
                                wpj[ct][:],
                                start=(ct == 0), stop=(ct == CH // 128 - 1))
                        ys = ysbp.tile([128, 512], F32, tag="ys")
                        nc.scalar.copy(ys[:], accp[:])
                        nc.sync.dma_start(
                            out=y[tt * 128:(tt + 1) * 128,
                                  nf * 512:(nf + 1) * 512],
                            in_=ys[:])

    nc.compile()
    return nc


def _make_masks():
    k = np.arange(128)[:, None, None]
    j = np.arange(4)[None, :, None]
    q = np.arange(512)[None, None, :]
    m = (j * 128 + k <= q)                       # [128, 4, 512]
    m2 = np.concatenate([m, m], axis=2)          # [128, 4, 1024] (both heads)
    return m2.astype(np.float32)


def make_in_maps(x, w_qkv, w_proj):
    masks = _make_masks()
    ones = np.ones((128, 64), dtype=np.float32)
    in_maps = []
    for c in range(N_CORES):
        b, g = c // 2, c % 2
        xTv = np.ascontiguousarray(x[b].T)
        wq = w_qkv[:, g * CH:(g + 1) * CH]
        wk = w_qkv[:, C + g * CH:C + (g + 1) * CH]
        stacked = np.concatenate([wq, wk], axis=1)           # [1024, 1024]
        # [ft, c_within_tile, kc, f]: tile ft, contraction row c of chunk
        # kc, feature f -> stacked[kc*128 + c, ft*128 + f]
        w_qk = np.ascontiguousarray(
            stacked.reshape(KC, 128, FT, 128).transpose(2, 1, 0, 3))
        w_v = np.ascontiguousarray(
            w_qkv[:, 2 * C + g * CH:2 * C + (g + 1) * CH].reshape(KC, 128, CH))
        w_pj = np.ascontiguousarray(
            w_proj[g * CH:(g + 1) * CH, :].reshape(CH // 128, 128, C))
        in_maps.append({
            "xT": xTv, "w_qk": w_qk, "w_v": w_v, "w_pj": w_pj,
            "masks": masks, "ones_d": ones,
        })
    return in_maps


def kernel(x, w_qkv, w_proj):
    x = np.asarray(x, dtype=np.float32)
    w_qkv = np.asarray(w_qkv, dtype=np.float32)
    w_proj = np.asarray(w_proj, dtype=np.float32)

    if "nc" not in _CACHE:
        _CACHE["nc"] = build_kernel()
    nc = _CACHE["nc"]

    in_maps = make_in_maps(x, w_qkv, w_proj)
    res = run_bass_kernel_spmd(nc, in_maps, core_ids=list(range(N_CORES)))
    _CACHE["last_result"] = res

    y = np.empty((B, T, C), dtype=np.float32)
    for b in range(B):
        y[b] = res.results[2 * b]["y"] + res.results[2 * b + 1]["y"]
    return y



# revision 7
# speedup vs baseline: 1.4860x; 1.4860x over previous
"""Causal self-attention on 8 trn2 NeuronCores (v2, bf16 pipelined).

Sharding: core c -> (batch b = c//2, head-group g = c%2 of 8 heads).
Each core computes qkv for its (batch, head-group), causal attention for
its 8 heads, and the row-slice of the output projection for its 512
channels. Host sums the two per-batch partial projections (bf16 partials
upcast to f32).

v2 design (vs v1):
- Full bf16 datapath (PE rate is 1 cycle/row for bf16 = same as f32r,
  but DMA halves, DVE gets 2x modes, and narrow matmuls lose the f32r
  <256-wide 4x penalty).
- x and all weights SBUF-resident, loaded once (v1 re-streamed w_qk 4x).
- Single kernel-lifetime PSUM pools sized to exactly 8 banks:
  acc(2 slots x 1 bank) + sw(2 x 2 banks) + o0/o1(1 bank each) -- v1's
  scoped per-phase pools serialized phases on pool release.
- Software-pipelined emission: Q(0); A(0) Q(1); A(1) P(0) Q(2); ... so
  the tensor engine always has ready work while exp runs on Act.
- Normalization: reciprocal straight from PSUM (denominator from the
  ones-column of V inside the PV matmul), gpsimd partition-broadcast,
  one DVE multiply per head fused with the PSUM->SBUF copy.
- Causal: off-diagonal blocks run unmasked, blocks above the diagonal
  are skipped, diagonal blocks get one narrow [128,2,128] triangular
  0/1 mask multiply (the triangle is identical for every j).
"""

import sys

sys.path.insert(0, "/opt/trn_rl_repo")

import numpy as np
import ml_dtypes

import concourse.bass as bass
import concourse.mybir as mybir
import concourse.tile as tile
from concourse import bacc
from concourse.bass_utils import run_bass_kernel_spmd

# Problem shape (hardcoded per the contract).
B = 4
T = 2048
C = 1024
N_HEADS = 16
HD = 64
N_CORES = 8

# Per-core sharding.
H_PER_CORE = 8          # heads per core
CH = H_PER_CORE * HD    # 512 channels per core
KC = C // 128           # 8 contraction tiles over model dim
FT = CH * 2 // 128      # 8 feature tiles for q+k ([q0..q3, k0..k3])
TT = T // 128           # 16 token tiles
QM = T // 512           # 4 query macros / token quarters
SCALE = HD ** -0.5

F32 = mybir.dt.float32
BF16 = mybir.dt.bfloat16

_CACHE = {}


def build_kernel(debug=False):
    nc = bacc.Bacc(target_bir_lowering=False)

    xT_d = nc.dram_tensor("xT", [C, T], BF16, kind="ExternalInput")
    wqk_d = nc.dram_tensor("w_qk", [FT, 128, KC, 128], BF16, kind="ExternalInput")
    wv_d = nc.dram_tensor("w_v", [KC, 128, CH], BF16, kind="ExternalInput")
    wpj_d = nc.dram_tensor("w_pj", [CH // 128, 128, C], BF16, kind="ExternalInput")
    mask_d = nc.dram_tensor("masks", [128, 2, 128], BF16, kind="ExternalInput")
    y = nc.dram_tensor("y", [T, C], BF16, kind="ExternalOutput")
    if debug:
        dbg_qk = nc.dram_tensor("dbg_qk", [FT, 128, T], BF16,
                                kind="ExternalOutput")
        dbg_v = nc.dram_tensor("dbg_v", [TT, 128, H_PER_CORE, HD + 1], BF16,
                               kind="ExternalOutput")
        dbg_o = nc.dram_tensor("dbg_o", [CH // 128, 128, T], BF16,
                               kind="ExternalOutput")
        dbg_den = nc.dram_tensor("dbg_den", [4, 2, 2, 1, 512], F32,
                                 kind="ExternalOutput")

    with tile.TileContext(nc) as tc:
        with (
            tc.tile_pool(name="big", bufs=1) as big,
            tc.tile_pool(name="pacc", bufs=2, space="PSUM") as pacc,
            tc.tile_pool(name="psw", bufs=2, space="PSUM") as psw,
            tc.tile_pool(name="pso", bufs=1, space="PSUM") as pso,
            tc.tile_pool(name="ptp", bufs=3) as ptp,
            tc.tile_pool(name="sml", bufs=2) as sml,
            tc.tile_pool(name="ysp", bufs=3) as ysp,
        ):
            # ---- resident SBUF tensors ----
            xsb = [big.tile([128, T], BF16, tag=f"x{kc}", name=f"x{kc}")
                   for kc in range(KC)]
            wqk = [big.tile([128, KC, 128], BF16, tag=f"wqk{ft}",
                            name=f"wqk{ft}") for ft in range(FT)]
            wv = [big.tile([128, CH], BF16, tag=f"wv{kc}", name=f"wv{kc}")
                  for kc in range(KC)]
            wpj = [big.tile([128, C], BF16, tag=f"wpj{ct}", name=f"wpj{ct}")
                   for ct in range(CH // 128)]
            mask_sb = big.tile([128, 2, 128], BF16, tag="mask", name="mask")
            qkT = [big.tile([128, T], BF16, tag=f"qkT{ft}", name=f"qkT{ft}")
                   for ft in range(FT)]
            vt = [big.tile([128, H_PER_CORE, HD + 1], BF16, tag=f"v{tt}",
                           name=f"v{tt}") for tt in range(TT)]
            outT = [big.tile([128, T], BF16, tag=f"outT{ct}", name=f"outT{ct}")
                    for ct in range(CH // 128)]

            # ---- preload ----
            for kc in range(KC):
                nc.sync.dma_start(out=xsb[kc],
                                  in_=xT_d[kc * 128:(kc + 1) * 128, :])
            for ft in range(FT):
                nc.sync.dma_start(out=wqk[ft], in_=wqk_d[ft])
            for kc in range(KC):
                nc.sync.dma_start(out=wv[kc], in_=wv_d[kc])
            for ct in range(CH // 128):
                nc.sync.dma_start(out=wpj[ct], in_=wpj_d[ct])
            nc.sync.dma_start(out=mask_sb, in_=mask_d[:])
            for tt in range(TT):
                nc.gpsimd.memset(vt[tt][:, :, HD:HD + 1], 1.0)

            # ---- phase Q: qkv projections for token quarter tq ----
            def phase_q(tq):
                lo, hi = tq * 512, (tq + 1) * 512
                for ft in range(FT):
                    acc = pacc.tile([128, 512], F32, tag="acc",
                                    name=f"qk{tq}_{ft}")
                    for kc in range(KC):
                        nc.tensor.matmul(
                            acc[:], wqk[ft][:, kc, :], xsb[kc][:, lo:hi],
                            start=(kc == 0), stop=(kc == KC - 1))
                    nc.vector.tensor_copy(qkT[ft][:, lo:hi], acc[:])
                for i in range(4):
                    tt = tq * 4 + i
                    vac = pacc.tile([128, CH], F32, tag="acc",
                                    name=f"vac{tt}")
                    for kc in range(KC):
                        nc.tensor.matmul(
                            vac[:],
                            xsb[kc][:, tt * 128:(tt + 1) * 128],
                            wv[kc][:],
                            start=(kc == 0), stop=(kc == KC - 1))
                    nc.vector.tensor_copy(
                        vt[tt][:, :, 0:HD],
                        vac[:].rearrange("p (h d) -> p h d", h=H_PER_CORE))

            # ---- phase A: causal attention for query macro qm ----
            def phase_a(qm):
                nkt = 4 * qm + 4
                for p in range(4):          # head pair = heads 2p, 2p+1
                    qTh = qkT[p]
                    kTh = qkT[4 + p]
                    oacc = [pso.tile([HD + 1, 512], F32, tag=f"o{hh}",
                                     name=f"o{p}_{qm}_{hh}") for hh in range(2)]
                    for kt in range(nkt):
                        j = kt - 4 * qm     # >=0 on diagonal blocks
                        o0 = max(j, 0) * 128
                        sw = psw.tile([128, 1024], F32, tag="sw", name="sw")
                        for hh in range(2):
                            nc.tensor.matmul(
                                sw[:, hh * 512 + o0:(hh + 1) * 512],
                                kTh[hh * 64:(hh + 1) * 64,
                                    kt * 128:(kt + 1) * 128],
                                qTh[hh * 64:(hh + 1) * 64,
                                    qm * 512 + o0:(qm + 1) * 512],
                                start=True, stop=True)
                        pt = ptp.tile([128, 1024], BF16, tag="pt", name="pt")
                        swv = sw[:].rearrange("p (a q) -> p a q", a=2)
                        ptv = pt[:].rearrange("p (a q) -> p a q", a=2)
                        nc.scalar.activation(
                            ptv[:, :, o0:512], swv[:, :, o0:512],
                            mybir.ActivationFunctionType.Exp, scale=SCALE)
                        if j >= 0:      # diagonal: triangular mask, both heads
                            nc.vector.tensor_mul(
                                ptv[:, :, o0:o0 + 128],
                                ptv[:, :, o0:o0 + 128],
                                mask_sb[:])
                        for hh in range(2):
                            h = 2 * p + hh
                            nc.tensor.matmul(
                                oacc[hh][:, o0:512],
                                vt[kt][:, h, :],
                                pt[:, hh * 512 + o0:(hh + 1) * 512],
                                start=(kt == 0), stop=(kt == nkt - 1),
                                skip_group_check=True)
                    rcps = [sml.tile([1, 512], F32, tag=f"rcp{hh}",
                                     name=f"rcp{hh}") for hh in range(2)]
                    dens = [sml.tile([1, 512], F32, tag=f"den{hh}",
                                     name=f"den{hh}") for hh in range(2)]
                    for hh in range(2):
                        nc.vector.tensor_copy(dens[hh][:],
                                              oacc[hh][HD:HD + 1, :])
                        nc.vector.reciprocal_approx_fast(
                            rcps[hh][:], dens[hh][:])
                    if debug and qm == 3:
                        for hh in range(2):
                            nc.sync.dma_start(out=dbg_den[p, 0, hh],
                                              in_=dens[hh][:])
                            nc.sync.dma_start(out=dbg_den[p, 1, hh],
                                              in_=rcps[hh][:])
                    for hh in range(2):
                        bcs = sml.tile([HD, 512], F32, tag=f"bcs{hh}",
                                       name=f"bcs{hh}")
                        nc.gpsimd.partition_broadcast(bcs[:], rcps[hh][:])
                        nc.vector.tensor_mul(
                            outT[p][hh * 64:(hh + 1) * 64,
                                    qm * 512:(qm + 1) * 512],
                            oacc[hh][0:HD, :], bcs[:])

            # ---- phase P: output projection for token quarter qm ----
            def phase_p(qm):
                for i in range(4):
                    tt = qm * 4 + i
                    for nf in range(2):
                        accp = pacc.tile([128, 512], F32, tag="acc",
                                         name=f"pp{tt}_{nf}")
                        for ct in range(CH // 128):
                            nc.tensor.matmul(
                                accp[:],
                                outT[ct][:, tt * 128:(tt + 1) * 128],
                                wpj[ct][:, nf * 512:(nf + 1) * 512],
                                start=(ct == 0), stop=(ct == CH // 128 - 1))
                        ys = ysp.tile([128, 512], BF16, tag="ys", name="ys")
                        nc.vector.tensor_copy(ys[:], accp[:])
                        nc.sync.dma_start(
                            out=y[tt * 128:(tt + 1) * 128,
                                  nf * 512:(nf + 1) * 512],
                            in_=ys[:])

            # ---- software-pipelined emission ----
            phase_q(0)
            phase_a(0)
            phase_q(1)
            phase_a(1)
            phase_p(0)
            phase_q(2)
            phase_a(2)
            phase_p(1)
            phase_q(3)
            phase_a(3)
            phase_p(2)
            phase_p(3)

            if debug:
                for ft in range(FT):
                    nc.sync.dma_start(out=dbg_qk[ft], in_=qkT[ft][:])
                for tt in range(TT):
                    nc.sync.dma_start(out=dbg_v[tt], in_=vt[tt][:])
                for ct in range(CH // 128):
                    nc.sync.dma_start(out=dbg_o[ct], in_=outT[ct][:])

    nc.compile()
    return nc


def _make_masks():
    k = np.arange(128)[:, None, None]
    q = np.arange(128)[None, None, :]
    m = (k <= q) | np.zeros((1, 2, 1), dtype=bool)   # [128, 2, 128]
    return m.astype(ml_dtypes.bfloat16)


def make_in_maps(x, w_qkv, w_proj):
    masks = _make_masks()
    bf = ml_dtypes.bfloat16
    in_maps = []
    for c in range(N_CORES):
        b, g = c // 2, c % 2
        xTv = np.ascontiguousarray(x[b].T.astype(bf))
        wq = w_qkv[:, g * CH:(g + 1) * CH]
        wk = w_qkv[:, C + g * CH:C + (g + 1) * CH]
        stacked = np.concatenate([wq, wk], axis=1)           # [1024, 1024]
        # [ft, c_within_tile, kc, f]: tile ft, contraction row c of chunk
        # kc, feature f -> stacked[kc*128 + c, ft*128 + f]
        w_qk = np.ascontiguousarray(
            stacked.reshape(KC, 128, FT, 128).transpose(2, 1, 0, 3).astype(bf))
        w_v = np.ascontiguousarray(
            w_qkv[:, 2 * C + g * CH:2 * C + (g + 1) * CH]
            .reshape(KC, 128, CH).astype(bf))
        w_pj = np.ascontiguousarray(
            w_proj[g * CH:(g + 1) * CH, :].reshape(CH // 128, 128, C)
            .astype(bf))
        in_maps.append({
            "xT": xTv, "w_qk": w_qk, "w_v": w_v, "w_pj": w_pj,
            "masks": masks,
        })
    return in_maps


def kernel(x, w_qkv, w_proj):
    x = np.asarray(x, dtype=np.float32)
    w_qkv = np.asarray(w_qkv, dtype=np.float32)
    w_proj = np.asarray(w_proj, dtype=np.float32)

    if "nc" not in _CACHE:
        _CACHE["nc"] = build_kernel()
    nc = _CACHE["nc"]

    in_maps = make_in_maps(x, w_qkv, w_proj)
    res = run_bass_kernel_spmd(nc, in_maps, core_ids=list(range(N_CORES)))
    _CACHE["last_result"] = res

    y = np.empty((B, T, C), dtype=np.float32)
    for b in range(B):
        y[b] = (res.results[2 * b]["y"].astype(np.float32)
                + res.results[2 * b + 1]["y"].astype(np.float32))
    return y


# revision 10
# speedup vs baseline: 1.5506x; 1.0435x over previous
"""Causal self-attention on 8 trn2 NeuronCores (v2, bf16 pipelined).

Sharding: core c -> (batch b = c//2, head-group g = c%2 of 8 heads).
Each core computes qkv for its (batch, head-group), causal attention for
its 8 heads, and the row-slice of the output projection for its 512
channels. Host sums the two per-batch partial projections (bf16 partials
upcast to f32).

v2 design (vs v1):
- Full bf16 datapath (PE rate is 1 cycle/row for bf16 = same as f32r,
  but DMA halves, DVE gets 2x modes, and narrow matmuls lose the f32r
  <256-wide 4x penalty).
- x and all weights SBUF-resident, loaded once (v1 re-streamed w_qk 4x).
- Single kernel-lifetime PSUM pools sized to exactly 8 banks:
  acc(2 slots x 1 bank) + sw(2 x 2 banks) + o0/o1(1 bank each) -- v1's
  scoped per-phase pools serialized phases on pool release.
- Software-pipelined emission: Q(0); A(0) Q(1); A(1) P(0) Q(2); ... so
  the tensor engine always has ready work while exp runs on Act.
- Normalization: reciprocal straight from PSUM (denominator from the
  ones-column of V inside the PV matmul), gpsimd partition-broadcast,
  one DVE multiply per head fused with the PSUM->SBUF copy.
- Causal: off-diagonal blocks run unmasked, blocks above the diagonal
  are skipped, diagonal blocks get one narrow [128,2,128] triangular
  0/1 mask multiply (the triangle is identical for every j).
"""

import sys

sys.path.insert(0, "/opt/trn_rl_repo")

import numpy as np
import ml_dtypes

import concourse.bass as bass
import concourse.mybir as mybir
import concourse.tile as tile
from concourse import bacc
from concourse.bass_utils import run_bass_kernel_spmd

# Problem shape (hardcoded per the contract).
B = 4
T = 2048
C = 1024
N_HEADS = 16
HD = 64
N_CORES = 8

# Per-core sharding.
H_PER_CORE = 8          # heads per core
CH = H_PER_CORE * HD    # 512 channels per core
KC = C // 128           # 8 contraction tiles over model dim
FT = CH * 2 // 128      # 8 feature tiles for q+k ([q0..q3, k0..k3])
TT = T // 128           # 16 token tiles
QM = T // 512           # 4 query macros / token quarters
SCALE = HD ** -0.5

F32 = mybir.dt.float32
BF16 = mybir.dt.bfloat16

_CACHE = {}


def build_kernel(debug=False):
    nc = bacc.Bacc(target_bir_lowering=False)

    xT_d = nc.dram_tensor("xT", [C, T], BF16, kind="ExternalInput")
    wqk_d = nc.dram_tensor("w_qk", [FT, 128, KC, 128], BF16, kind="ExternalInput")
    wv_d = nc.dram_tensor("w_v", [KC, 128, CH], BF16, kind="ExternalInput")
    wpj_d = nc.dram_tensor("w_pj", [CH // 128, 128, C], BF16, kind="ExternalInput")
    mask_d = nc.dram_tensor("masks", [128, 2, 128], BF16, kind="ExternalInput")
    y = nc.dram_tensor("y", [T, C], BF16, kind="ExternalOutput")
    if debug:
        dbg_qk = nc.dram_tensor("dbg_qk", [FT, 128, T], BF16,
                                kind="ExternalOutput")
        dbg_v = nc.dram_tensor("dbg_v", [TT, 128, H_PER_CORE, HD + 1], BF16,
                               kind="ExternalOutput")
        dbg_o = nc.dram_tensor("dbg_o", [CH // 128, 128, T], BF16,
                               kind="ExternalOutput")
        dbg_den = nc.dram_tensor("dbg_den", [4, 2, 2, 1, 512], F32,
                                 kind="ExternalOutput")

    with tile.TileContext(nc) as tc:
        with (
            tc.tile_pool(name="big", bufs=1) as big,
            tc.tile_pool(name="pacc", bufs=2, space="PSUM") as pacc,
            tc.tile_pool(name="psw", bufs=2, space="PSUM") as psw,
            tc.tile_pool(name="pso", bufs=1, space="PSUM") as pso,
            tc.tile_pool(name="ptp", bufs=3) as ptp,
            tc.tile_pool(name="sml", bufs=2) as sml,
            tc.tile_pool(name="ysp", bufs=3) as ysp,
        ):
            # ---- resident SBUF tensors ----
            xsb = [big.tile([128, T], BF16, tag=f"x{kc}", name=f"x{kc}")
                   for kc in range(KC)]
            wqk = [big.tile([128, KC, 128], BF16, tag=f"wqk{ft}",
                            name=f"wqk{ft}") for ft in range(FT)]
            wv = [big.tile([128, CH], BF16, tag=f"wv{kc}", name=f"wv{kc}")
                  for kc in range(KC)]
            wpj = [big.tile([128, C], BF16, tag=f"wpj{ct}", name=f"wpj{ct}")
                   for ct in range(CH // 128)]
            mask_sb = big.tile([128, 2, 128], BF16, tag="mask", name="mask")
            qkT = [big.tile([128, T], BF16, tag=f"qkT{ft}", name=f"qkT{ft}")
                   for ft in range(FT)]
            vt = [big.tile([128, H_PER_CORE, HD + 1], BF16, tag=f"v{tt}",
                           name=f"v{tt}") for tt in range(TT)]
            outT = [big.tile([128, T], BF16, tag=f"outT{ct}", name=f"outT{ct}")
                    for ct in range(CH // 128)]

            # ---- preload (use-ordered; gpsimd sequencer has the cheapest
            # DMA trigger, and Q(0) only needs token quarter 0 of x) ----
            for kc in range(KC):
                nc.gpsimd.dma_start(out=xsb[kc][:, 0:512],
                                    in_=xT_d[kc * 128:(kc + 1) * 128, 0:512])
            for ft in range(FT):
                nc.gpsimd.dma_start(out=wqk[ft], in_=wqk_d[ft])
            for kc in range(KC):
                nc.gpsimd.dma_start(out=wv[kc], in_=wv_d[kc])
            nc.gpsimd.dma_start(out=mask_sb, in_=mask_d[:])
            for tq in range(1, 4):
                for kc in range(KC):
                    nc.gpsimd.dma_start(
                        out=xsb[kc][:, tq * 512:(tq + 1) * 512],
                        in_=xT_d[kc * 128:(kc + 1) * 128,
                                 tq * 512:(tq + 1) * 512])
            for ct in range(CH // 128):
                nc.gpsimd.dma_start(out=wpj[ct], in_=wpj_d[ct])
            for tt in range(TT):
                nc.gpsimd.memset(vt[tt][:, :, HD:HD + 1], 1.0)

            # ---- phase Q: qkv projections for token quarter tq ----
            def phase_q(tq):
                lo, hi = tq * 512, (tq + 1) * 512
                for ft in range(FT):
                    acc = pacc.tile([128, 512], F32, tag="acc",
                                    name=f"qk{tq}_{ft}")
                    for kc in range(KC):
                        nc.tensor.matmul(
                            acc[:], wqk[ft][:, kc, :], xsb[kc][:, lo:hi],
                            start=(kc == 0), stop=(kc == KC - 1))
                    nc.vector.tensor_copy(qkT[ft][:, lo:hi], acc[:])
                for i in range(4):
                    tt = tq * 4 + i
                    vac = pacc.tile([128, CH], F32, tag="acc",
                                    name=f"vac{tt}")
                    for kc in range(KC):
                        nc.tensor.matmul(
                            vac[:],
                            xsb[kc][:, tt * 128:(tt + 1) * 128],
                            wv[kc][:],
                            start=(kc == 0), stop=(kc == KC - 1))
                    nc.vector.tensor_copy(
                        vt[tt][:, :, 0:HD],
                        vac[:].rearrange("p (h d) -> p h d", h=H_PER_CORE))

            # ---- phase A: causal attention for query macro qm ----
            def phase_a(qm, pairs=(0, 1, 2, 3)):
                nkt = 4 * qm + 4
                for p in pairs:             # head pair = heads 2p, 2p+1
                    qTh = qkT[p]
                    kTh = qkT[4 + p]
                    oacc = [pso.tile([HD + 1, 512], F32, tag=f"o{hh}",
                                     name=f"o{p}_{qm}_{hh}") for hh in range(2)]
                    for kt in range(nkt):
                        j = kt - 4 * qm     # >=0 on diagonal blocks
                        o0 = max(j, 0) * 128
                        sw = psw.tile([128, 1024], F32, tag="sw", name="sw")
                        for hh in range(2):
                            nc.tensor.matmul(
                                sw[:, hh * 512 + o0:(hh + 1) * 512],
                                kTh[hh * 64:(hh + 1) * 64,
                                    kt * 128:(kt + 1) * 128],
                                qTh[hh * 64:(hh + 1) * 64,
                                    qm * 512 + o0:(qm + 1) * 512],
                                start=True, stop=True)
                        pt = ptp.tile([128, 1024], BF16, tag="pt", name="pt")
                        swv = sw[:].rearrange("p (a q) -> p a q", a=2)
                        ptv = pt[:].rearrange("p (a q) -> p a q", a=2)
                        nc.scalar.activation(
                            ptv[:, :, o0:512], swv[:, :, o0:512],
                            mybir.ActivationFunctionType.Exp, scale=SCALE)
                        if j >= 0:      # diagonal: triangular mask, both heads
                            nc.vector.tensor_mul(
                                ptv[:, :, o0:o0 + 128],
                                ptv[:, :, o0:o0 + 128],
                                mask_sb[:])
                        for hh in range(2):
                            h = 2 * p + hh
                            nc.tensor.matmul(
                                oacc[hh][:, o0:512],
                                vt[kt][:, h, :],
                                pt[:, hh * 512 + o0:(hh + 1) * 512],
                                start=(kt == 0), stop=(kt == nkt - 1),
                                skip_group_check=True)
                    rcps = [sml.tile([1, 512], F32, tag=f"rcp{hh}",
                                     name=f"rcp{hh}") for hh in range(2)]
                    dens = [sml.tile([1, 512], F32, tag=f"den{hh}",
                                     name=f"den{hh}") for hh in range(2)]
                    for hh in range(2):
                        nc.vector.tensor_copy(dens[hh][:],
                                              oacc[hh][HD:HD + 1, :])
                        nc.vector.reciprocal_approx_fast(
                            rcps[hh][:], dens[hh][:])
                    if debug and qm == 3:
                        for hh in range(2):
                            nc.sync.dma_start(out=dbg_den[p, 0, hh],
                                              in_=dens[hh][:])
                            nc.sync.dma_start(out=dbg_den[p, 1, hh],
                                              in_=rcps[hh][:])
                    for hh in range(2):
                        bcs = sml.tile([HD, 512], F32, tag=f"bcs{hh}",
                                       name=f"bcs{hh}")
                        nc.gpsimd.partition_broadcast(bcs[:], rcps[hh][:])
                        nc.vector.tensor_mul(
                            outT[p][hh * 64:(hh + 1) * 64,
                                    qm * 512:(qm + 1) * 512],
                            oacc[hh][0:HD, :], bcs[:])

            # ---- phase P: output projection for token quarter qm ----
            def phase_p(qm):
                for i in range(4):
                    tt = qm * 4 + i
                    for nf in range(2):
                        accp = pacc.tile([128, 512], F32, tag="acc",
                                         name=f"pp{tt}_{nf}")
                        for ct in range(CH // 128):
                            nc.tensor.matmul(
                                accp[:],
                                outT[ct][:, tt * 128:(tt + 1) * 128],
                                wpj[ct][:, nf * 512:(nf + 1) * 512],
                                start=(ct == 0), stop=(ct == CH // 128 - 1))
                        ys = ysp.tile([128, 512], BF16, tag="ys", name="ys")
                        nc.vector.tensor_copy(ys[:], accp[:])
                        nc.sync.dma_start(
                            out=y[tt * 128:(tt + 1) * 128,
                                  nf * 512:(nf + 1) * 512],
                            in_=ys[:])

            # ---- software-pipelined emission ----
            # Q(tq+1) fills tensor-engine gaps while A(tq)'s exp runs on
            # the Act engine; all of P is deferred to the endgame where
            # A(3) would otherwise be exp-latency-bound with no filler.
            phase_q(0)
            phase_a(0)
            phase_q(1)
            phase_a(1)
            phase_q(2)
            phase_a(2)
            phase_q(3)
            phase_a(3, pairs=(0,))
            phase_p(0)
            phase_a(3, pairs=(1,))
            phase_p(1)
            phase_a(3, pairs=(2,))
            phase_p(2)
            phase_a(3, pairs=(3,))
            phase_p(3)

            if debug:
                for ft in range(FT):
                    nc.sync.dma_start(out=dbg_qk[ft], in_=qkT[ft][:])
                for tt in range(TT):
                    nc.sync.dma_start(out=dbg_v[tt], in_=vt[tt][:])
                for ct in range(CH // 128):
                    nc.sync.dma_start(out=dbg_o[ct], in_=outT[ct][:])

    nc.compile()
    return nc


def _make_masks():
    k = np.arange(128)[:, None, None]
    q = np.arange(128)[None, None, :]
    m = (k <= q) | np.zeros((1, 2, 1), dtype=bool)   # [128, 2, 128]
    return m.astype(ml_dtypes.bfloat16)


def make_in_maps(x, w_qkv, w_proj):
    masks = _make_masks()
    bf = ml_dtypes.bfloat16
    in_maps = []
    for c in range(N_CORES):
        b, g = c // 2, c % 2
        xTv = np.ascontiguousarray(x[b].T.astype(bf))
        wq = w_qkv[:, g * CH:(g + 1) * CH]
        wk = w_qkv[:, C + g * CH:C + (g + 1) * CH]
        stacked = np.concatenate([wq, wk], axis=1)           # [1024, 1024]
        # [ft, c_within_tile, kc, f]: tile ft, contraction row c of chunk
        # kc, feature f -> stacked[kc*128 + c, ft*128 + f]
        w_qk = np.ascontiguousarray(
            stacked.reshape(KC, 128, FT, 128).transpose(2, 1, 0, 3).astype(bf))
        w_v = np.ascontiguousarray(
            w_qkv[:, 2 * C + g * CH:2 * C + (g + 1) * CH]
            .reshape(KC, 128, CH).astype(bf))
        w_pj = np.ascontiguousarray(
            w_proj[g * CH:(g + 1) * CH, :].reshape(CH // 128, 128, C)
            .astype(bf))
        in_maps.append({
            "xT": xTv, "w_qk": w_qk, "w_v": w_v, "w_pj": w_pj,
            "masks": masks,
        })
    return in_maps


def kernel(x, w_qkv, w_proj):
    x = np.asarray(x, dtype=np.float32)
    w_qkv = np.asarray(w_qkv, dtype=np.float32)
    w_proj = np.asarray(w_proj, dtype=np.float32)

    if "nc" not in _CACHE:
        _CACHE["nc"] = build_kernel()
    nc = _CACHE["nc"]

    in_maps = make_in_maps(x, w_qkv, w_proj)
    res = run_bass_kernel_spmd(nc, in_maps, core_ids=list(range(N_CORES)))
    _CACHE["last_result"] = res

    y = np.empty((B, T, C), dtype=np.float32)
    for b in range(B):
        y[b] = (res.results[2 * b]["y"].astype(np.float32)
                + res.results[2 * b + 1]["y"].astype(np.float32))
    return y


# revision 13
# speedup vs baseline: 1.6144x; 1.0412x over previous
"""Causal self-attention on 8 trn2 NeuronCores (v2, bf16 pipelined).

Sharding: core c -> (batch b = c//2, head-group g = c%2 of 8 heads).
Each core computes qkv for its (batch, head-group), causal attention for
its 8 heads, and the row-slice of the output projection for its 512
channels. Host sums the two per-batch partial projections (bf16 partials
upcast to f32).

v2 design (vs v1):
- Full bf16 datapath (PE rate is 1 cycle/row for bf16 = same as f32r,
  but DMA halves, DVE gets 2x modes, and narrow matmuls lose the f32r
  <256-wide 4x penalty).
- x and all weights SBUF-resident, loaded once (v1 re-streamed w_qk 4x).
- Single kernel-lifetime PSUM pools sized to exactly 8 banks:
  acc(2 slots x 1 bank) + sw(2 x 2 banks) + o0/o1(1 bank each) -- v1's
  scoped per-phase pools serialized phases on pool release.
- Software-pipelined emission: Q(0); A(0) Q(1); A(1) P(0) Q(2); ... so
  the tensor engine always has ready work while exp runs on Act.
- Normalization: reciprocal straight from PSUM (denominator from the
  ones-column of V inside the PV matmul), gpsimd partition-broadcast,
  one DVE multiply per head fused with the PSUM->SBUF copy.
- Causal: off-diagonal blocks run unmasked, blocks above the diagonal
  are skipped, diagonal blocks get one narrow [128,2,128] triangular
  0/1 mask multiply (the triangle is identical for every j).
"""

import sys

sys.path.insert(0, "/opt/trn_rl_repo")

import numpy as np
import ml_dtypes

import concourse.bass as bass
import concourse.mybir as mybir
import concourse.tile as tile
from concourse import bacc
from concourse.bass_utils import run_bass_kernel_spmd

# Problem shape (hardcoded per the contract).
B = 4
T = 2048
C = 1024
N_HEADS = 16
HD = 64
N_CORES = 8

# Per-core sharding.
H_PER_CORE = 8          # heads per core
CH = H_PER_CORE * HD    # 512 channels per core
KC = C // 128           # 8 contraction tiles over model dim
FT = CH * 2 // 128      # 8 feature tiles for q+k ([q0..q3, k0..k3])
TT = T // 128           # 16 token tiles
QM = T // 512           # 4 query macros / token quarters
SCALE = HD ** -0.5

F32 = mybir.dt.float32
BF16 = mybir.dt.bfloat16

_CACHE = {}


def build_kernel(debug=False):
    nc = bacc.Bacc(target_bir_lowering=False)

    xT_d = nc.dram_tensor("xT", [C, T], BF16, kind="ExternalInput")
    wqk_d = nc.dram_tensor("w_qk", [FT, 128, KC, 128], BF16, kind="ExternalInput")
    wv_d = nc.dram_tensor("w_v", [KC, 128, CH], BF16, kind="ExternalInput")
    wpj_d = nc.dram_tensor("w_pj", [CH // 128, 128, C], BF16, kind="ExternalInput")
    mask_d = nc.dram_tensor("masks", [128, 2, 128], BF16, kind="ExternalInput")
    y = nc.dram_tensor("y", [T, C], BF16, kind="ExternalOutput")
    if debug:
        dbg_qk = nc.dram_tensor("dbg_qk", [FT, 128, T], BF16,
                                kind="ExternalOutput")
        dbg_v = nc.dram_tensor("dbg_v", [TT, 128, H_PER_CORE, HD + 1], BF16,
                               kind="ExternalOutput")
        dbg_o = nc.dram_tensor("dbg_o", [CH // 128, 128, T], BF16,
                               kind="ExternalOutput")
        dbg_den = nc.dram_tensor("dbg_den", [4, 2, 2, 1, 512], F32,
                                 kind="ExternalOutput")

    with tile.TileContext(nc) as tc:
        with (
            tc.tile_pool(name="big", bufs=1) as big,
            tc.tile_pool(name="pacc", bufs=2, space="PSUM") as pacc,
            tc.tile_pool(name="psw", bufs=2, space="PSUM") as psw,
            tc.tile_pool(name="pso", bufs=1, space="PSUM") as pso,
            tc.tile_pool(name="ptp", bufs=3) as ptp,
            tc.tile_pool(name="sml", bufs=2) as sml,
            tc.tile_pool(name="ysp", bufs=3) as ysp,
        ):
            # ---- resident SBUF tensors ----
            xsb = [big.tile([128, T], BF16, tag=f"x{kc}", name=f"x{kc}")
                   for kc in range(KC)]
            wqk = [big.tile([128, KC, 128], BF16, tag=f"wqk{ft}",
                            name=f"wqk{ft}") for ft in range(FT)]
            wv = [big.tile([128, CH], BF16, tag=f"wv{kc}", name=f"wv{kc}")
                  for kc in range(KC)]
            wpj = [big.tile([128, C], BF16, tag=f"wpj{ct}", name=f"wpj{ct}")
                   for ct in range(CH // 128)]
            mask_sb = big.tile([128, 2, 128], BF16, tag="mask", name="mask")
            qkT = [big.tile([128, T], BF16, tag=f"qkT{ft}", name=f"qkT{ft}")
                   for ft in range(FT)]
            vt = [big.tile([128, H_PER_CORE, HD + 1], BF16, tag=f"v{tt}",
                           name=f"v{tt}") for tt in range(TT)]
            outT = [big.tile([128, T], BF16, tag=f"outT{ct}", name=f"outT{ct}")
                    for ct in range(CH // 128)]

            # ---- preload (use-ordered, triggers split across the two
            # idlest sequencers; Q(0) only needs token quarter 0 of x) ----
            def pre_dma(idx, out, in_):
                eng = nc.gpsimd if idx % 2 == 0 else nc.sync
                eng.dma_start(out=out, in_=in_)

            for kc in range(KC):
                pre_dma(kc, xsb[kc][:, 0:512],
                        xT_d[kc * 128:(kc + 1) * 128, 0:512])
            for ft in range(FT):
                pre_dma(ft, wqk[ft], wqk_d[ft])
            for kc in range(KC):
                pre_dma(kc, wv[kc], wv_d[kc])
            nc.sync.dma_start(out=mask_sb, in_=mask_d[:])
            for tq in range(1, 4):
                for kc in range(KC):
                    pre_dma(kc, xsb[kc][:, tq * 512:(tq + 1) * 512],
                            xT_d[kc * 128:(kc + 1) * 128,
                                 tq * 512:(tq + 1) * 512])
            for ct in range(CH // 128):
                pre_dma(ct, wpj[ct], wpj_d[ct])
            for tt in range(TT):
                nc.gpsimd.memset(vt[tt][:, :, HD:HD + 1], 1.0)

            # ---- phase Q: qkv projections for token quarter tq ----
            # part p emits [ft p (q-pair p), ft 4+p (k-pair p), v tile p]
            # so A(tq-1, p) can interleave between parts.
            def phase_q(tq, part=None):
                lo, hi = tq * 512, (tq + 1) * 512
                parts = range(4) if part is None else (part,)
                for pp in parts:
                    for ft in (pp, 4 + pp):
                        acc = pacc.tile([128, 512], F32, tag="acc",
                                        name=f"qk{tq}_{ft}")
                        for kc in range(KC):
                            nc.tensor.matmul(
                                acc[:], wqk[ft][:, kc, :], xsb[kc][:, lo:hi],
                                start=(kc == 0), stop=(kc == KC - 1))
                        nc.vector.tensor_copy(qkT[ft][:, lo:hi], acc[:])
                    tt = tq * 4 + pp
                    vac = pacc.tile([128, CH], F32, tag="acc",
                                    name=f"vac{tt}")
                    for kc in range(KC):
                        nc.tensor.matmul(
                            vac[:],
                            xsb[kc][:, tt * 128:(tt + 1) * 128],
                            wv[kc][:],
                            start=(kc == 0), stop=(kc == KC - 1))
                    nc.vector.tensor_copy(
                        vt[tt][:, :, 0:HD],
                        vac[:].rearrange("p (h d) -> p h d", h=H_PER_CORE))

            # ---- phase A: causal attention for query macro qm ----
            def phase_a(qm, pairs=(0, 1, 2, 3)):
                nkt = 4 * qm + 4
                for p in pairs:             # head pair = heads 2p, 2p+1
                    qTh = qkT[p]
                    kTh = qkT[4 + p]
                    oacc = [pso.tile([HD + 1, 512], F32, tag=f"o{hh}",
                                     name=f"o{p}_{qm}_{hh}") for hh in range(2)]
                    for kt in range(nkt):
                        j = kt - 4 * qm     # >=0 on diagonal blocks
                        o0 = max(j, 0) * 128
                        sw = psw.tile([128, 1024], F32, tag="sw", name="sw")
                        for hh in range(2):
                            nc.tensor.matmul(
                                sw[:, hh * 512 + o0:(hh + 1) * 512],
                                kTh[hh * 64:(hh + 1) * 64,
                                    kt * 128:(kt + 1) * 128],
                                qTh[hh * 64:(hh + 1) * 64,
                                    qm * 512 + o0:(qm + 1) * 512],
                                start=True, stop=True)
                        pt = ptp.tile([128, 1024], BF16, tag="pt", name="pt")
                        swv = sw[:].rearrange("p (a q) -> p a q", a=2)
                        ptv = pt[:].rearrange("p (a q) -> p a q", a=2)
                        nc.scalar.activation(
                            ptv[:, :, o0:512], swv[:, :, o0:512],
                            mybir.ActivationFunctionType.Exp, scale=SCALE)
                        if j >= 0:      # diagonal: triangular mask, both heads
                            nc.vector.tensor_mul(
                                ptv[:, :, o0:o0 + 128],
                                ptv[:, :, o0:o0 + 128],
                                mask_sb[:])
                        for hh in range(2):
                            h = 2 * p + hh
                            nc.tensor.matmul(
                                oacc[hh][:, o0:512],
                                vt[kt][:, h, :],
                                pt[:, hh * 512 + o0:(hh + 1) * 512],
                                start=(kt == 0), stop=(kt == nkt - 1),
                                skip_group_check=True)
                    rcps = [sml.tile([1, 512], F32, tag=f"rcp{hh}",
                                     name=f"rcp{hh}") for hh in range(2)]
                    dens = [sml.tile([1, 512], F32, tag=f"den{hh}",
                                     name=f"den{hh}") for hh in range(2)]
                    for hh in range(2):
                        nc.vector.tensor_copy(dens[hh][:],
                                              oacc[hh][HD:HD + 1, :])
                        nc.vector.reciprocal_approx_fast(
                            rcps[hh][:], dens[hh][:])
                    if debug and qm == 3:
                        for hh in range(2):
                            nc.sync.dma_start(out=dbg_den[p, 0, hh],
                                              in_=dens[hh][:])
                            nc.sync.dma_start(out=dbg_den[p, 1, hh],
                                              in_=rcps[hh][:])
                    for hh in range(2):
                        bcs = sml.tile([HD, 512], F32, tag=f"bcs{hh}",
                                       name=f"bcs{hh}")
                        nc.gpsimd.partition_broadcast(bcs[:], rcps[hh][:])
                        nc.vector.tensor_mul(
                            outT[p][hh * 64:(hh + 1) * 64,
                                    qm * 512:(qm + 1) * 512],
                            oacc[hh][0:HD, :], bcs[:])

            # ---- phase P: output projection for token quarter qm ----
            def phase_p(qm):
                for i in range(4):
                    tt = qm * 4 + i
                    for nf in range(2):
                        accp = pacc.tile([128, 512], F32, tag="acc",
                                         name=f"pp{tt}_{nf}")
                        for ct in range(CH // 128):
                            nc.tensor.matmul(
                                accp[:],
                                outT[ct][:, tt * 128:(tt + 1) * 128],
                                wpj[ct][:, nf * 512:(nf + 1) * 512],
                                start=(ct == 0), stop=(ct == CH // 128 - 1))
                        ys = ysp.tile([128, 512], BF16, tag="ys", name="ys")
                        nc.vector.tensor_copy(ys[:], accp[:])
                        nc.sync.dma_start(
                            out=y[tt * 128:(tt + 1) * 128,
                                  nf * 512:(nf + 1) * 512],
                            in_=ys[:])

            # ---- software-pipelined emission ----
            # Q(tq+1) parts fill tensor-engine gaps between A(tq) pairs
            # (each A pair ends with an exp/norm latency chain); all of P
            # is deferred to the endgame where A(3) would otherwise be
            # exp-latency-bound with no filler.
            phase_q(0)
            for p in range(4):
                phase_a(0, pairs=(p,))
                phase_q(1, part=p)
            for p in range(4):
                phase_a(1, pairs=(p,))
                phase_q(2, part=p)
            for p in range(4):
                phase_a(2, pairs=(p,))
                phase_q(3, part=p)
            phase_a(3, pairs=(0,))
            phase_p(0)
            phase_a(3, pairs=(1,))
            phase_p(1)
            phase_a(3, pairs=(2,))
            phase_a(3, pairs=(3,))
            phase_p(2)
            phase_p(3)

            if debug:
                for ft in range(FT):
                    nc.sync.dma_start(out=dbg_qk[ft], in_=qkT[ft][:])
                for tt in range(TT):
                    nc.sync.dma_start(out=dbg_v[tt], in_=vt[tt][:])
                for ct in range(CH // 128):
                    nc.sync.dma_start(out=dbg_o[ct], in_=outT[ct][:])

    nc.compile()
    return nc


def _make_masks():
    k = np.arange(128)[:, None, None]
    q = np.arange(128)[None, None, :]
    m = (k <= q) | np.zeros((1, 2, 1), dtype=bool)   # [128, 2, 128]
    return m.astype(ml_dtypes.bfloat16)


def make_in_maps(x, w_qkv, w_proj):
    masks = _make_masks()
    bf = ml_dtypes.bfloat16
    in_maps = []
    for c in range(N_CORES):
        b, g = c // 2, c % 2
        xTv = np.ascontiguousarray(x[b].T.astype(bf))
        wq = w_qkv[:, g * CH:(g + 1) * CH]
        wk = w_qkv[:, C + g * CH:C + (g + 1) * CH]
        stacked = np.concatenate([wq, wk], axis=1)           # [1024, 1024]
        # [ft, c_within_tile, kc, f]: tile ft, contraction row c of chunk
        # kc, feature f -> stacked[kc*128 + c, ft*128 + f]
        w_qk = np.ascontiguousarray(
            stacked.reshape(KC, 128, FT, 128).transpose(2, 1, 0, 3).astype(bf))
        w_v = np.ascontiguousarray(
            w_qkv[:, 2 * C + g * CH:2 * C + (g + 1) * CH]
            .reshape(KC, 128, CH).astype(bf))
        w_pj = np.ascontiguousarray(
            w_proj[g * CH:(g + 1) * CH, :].reshape(CH // 128, 128, C)
            .astype(bf))
        in_maps.append({
            "xT": xTv, "w_qk": w_qk, "w_v": w_v, "w_pj": w_pj,
            "masks": masks,
        })
    return in_maps


def kernel(x, w_qkv, w_proj):
    x = np.asarray(x, dtype=np.float32)
    w_qkv = np.asarray(w_qkv, dtype=np.float32)
    w_proj = np.asarray(w_proj, dtype=np.float32)

    if "nc" not in _CACHE:
        _CACHE["nc"] = build_kernel()
    nc = _CACHE["nc"]

    in_maps = make_in_maps(x, w_qkv, w_proj)
    res = run_bass_kernel_spmd(nc, in_maps, core_ids=list(range(N_CORES)))
    _CACHE["last_result"] = res

    y = np.empty((B, T, C), dtype=np.float32)
    for b in range(B):
        y[b] = (res.results[2 * b]["y"].astype(np.float32)
                + res.results[2 * b + 1]["y"].astype(np.float32))
    return y
